# revision 20
# baseline (speedup 1.0000x reference)
"""Cross-attention Trainium2 kernel, 8 NeuronCores, head-parallel sharding.

Reference computation (fp32):
    q = x @ Wq; k = cond @ Wk; v = cond @ Wv        (per-head dh=40, 8 heads)
    attn = softmax(q k^T / sqrt(dh)); out = (attn v) @ Wo + bo

Sharding: 16 (batch, head) pairs across 8 cores -> core c handles batch c//4,
heads 2*(c%4), 2*(c%4)+1.  Each core computes a partial [S, D_MODEL] output
(its two heads' contribution through Wo, bf16); the host sums the 4 partials
per batch in fp32 and adds the bias.

Design notes (v1, engine-balanced):
  - All matmul operands bf16 (psum accumulate fp32); inputs DMA'd as bf16.
  - Scores computed transposed: S^T[keys, queries] = kT-chunk^T @ qT block.
  - exp of the scores is split across THREE engines: ACT runs exact Exp;
    DVE and GPSIMD(Pool) run a Schraudolph-style fast exp producing bf16
    BIT PATTERNS via one tensor_scalar each (i16 = trunc(x*A + B), bitcast
    to bf16 ~= exp(x*scale), ~1.8% rms).  The bias constant was tuned
    numerically; softmax + the output bias attenuate the error ~40x at the
    final output (measured ~2.4e-3 rel_l2 contribution).
  - AV runs in the natural [query, dh] orientation: stationary P^T chunk
    [128k x 128q], moving V-chunk [128k x 41] (col 40 = ones -> softmax
    denominator accumulates in the same psum).  41-row moving operand makes
    this 3.2x cheaper on PE than the transposed orientation.
  - V is projected directly into natural [key, dh] layout (stationary =
    cond chunk, moving = Wv), no transposes.
  - Normalization: per-query reciprocal ([128,4] per block) applied as the
    per-partition scalar of a DVE tensor_scalar during the psum->SBUF copy.
  - The normalized AV output [128q, 40] is PE-transposed back to [40, 128]
    to feed the output projection; the projection result streams to DRAM
    as bf16 partials.
  - Software-pipelined emission: K proj (head 0) is emitted first, Q head 0
    block 0 next, and the first attention block starts while V projection
    and the remaining Q/K work is drip-fed into gaps of the attention kc
    loop, keeping the exp engines busy from ~30us onward.
"""

import sys

for _p in ("/opt/trn_rl_repo", "/root/.axon_site/_ro/trn_rl_repo"):
    if _p not in sys.path:
        sys.path.append(_p)

import numpy as np

B, S, SK = 2, 4096, 4096
D_MODEL, D_COND, H, DH = 320, 768, 8, 40
NCORES = 8
KC = 128             # key chunk (psum partitions for scores)
NKC = SK // KC       # 32 key chunks
QB = 512             # query block (psum bank width in fp32)
NQB = S // QB        # 8 query blocks
NSC = QB // 128      # 4 query sub-chunks per block (AV psum partitions)
SCALE = DH ** -0.5

# Schraudolph fast-exp constants (bf16 bit patterns): i16 = trunc(x*A + B),
# bitcast(i16 as bf16) ~= exp(x*SCALE).  C=7.0 tuned numerically (rms 1.8%).
SCH_A = 128.0 * np.log2(np.e) * SCALE
SCH_B = 127.0 * 128.0 - 7.0

# Per-32-chunk exp engine assignment: A=ACT exact, D=DVE Schraudolph.
# (GPSIMD/Pool cannot read PSUM on TRN2, so only ACT and DVE can consume
# the score banks; they are the exp bottleneck and are balanced against
# their other per-block duties.)
QUOTA_A, QUOTA_D, QUOTA_P = 16, 16, 0
# pool sizing knobs (sim-sweepable)
SCP_BUFS, AVP_BUFS, TPP_BUFS, OPP_BUFS, PTP_BUFS = 5, 1, 1, 1, 2

_CACHE = {}


def _mk_engine_seq():
    slots = [None] * NKC
    for ch, q in (("P", QUOTA_P), ("D", QUOTA_D), ("A", QUOTA_A)):
        if q == 0:
            continue
        step = NKC / q
        for j in range(q):
            i = int(j * step + step / 2) % NKC
            while slots[i] is not None:
                i = (i + 1) % NKC
            slots[i] = ch
    return slots


def _build_nc():
    import concourse.mybir as mybir
    import concourse.tile as tile
    from concourse import bacc
    from concourse.alu_op_type import AluOpType

    F32 = mybir.dt.float32
    BF16 = mybir.dt.bfloat16
    I16 = mybir.dt.int16
    EXP = mybir.ActivationFunctionType.Exp
    COPYF = mybir.ActivationFunctionType.Copy

    ENG = _mk_engine_seq()

    nc = bacc.Bacc(None, target_bir_lowering=False)

    # DRAM inputs (bf16).  Layouts are packed host-side for single DMAs.
    xq_d = nc.dram_tensor("xq", [128, 3, S], BF16, kind="ExternalInput")
    cond_d = nc.dram_tensor("cond6", [128, 6, SK], BF16, kind="ExternalInput")
    wq_d = nc.dram_tensor("wq", [128, 3, 2 * DH], BF16, kind="ExternalInput")
    wkv_d = nc.dram_tensor("wkv", [128, 6, 4 * DH], BF16, kind="ExternalInput")
    wo_d = nc.dram_tensor("wo", [DH, 2, D_MODEL], BF16, kind="ExternalInput")
    eye_d = nc.dram_tensor("eyed", [128, 128], BF16, kind="ExternalInput")
    out_d = nc.dram_tensor("out", [S, D_MODEL], BF16, kind="ExternalOutput")

    XCH = [128, 128, 64]  # x feature chunks (320 total)

    with tile.TileContext(nc) as tc:
      with (
          tc.tile_pool(name="persist", bufs=1) as pp,
          tc.tile_pool(name="pt", bufs=PTP_BUFS) as ptp,
          tc.tile_pool(name="onat", bufs=2) as onp,
          tc.tile_pool(name="rcp", bufs=2) as rcp,
          tc.tile_pool(name="outT", bufs=6) as otp,
          tc.tile_pool(name="osb", bufs=2) as obp,
          tc.tile_pool(name="sps", bufs=SCP_BUFS, space="PSUM") as scp,
          tc.tile_pool(name="avps", bufs=AVP_BUFS, space="PSUM") as avp,
          tc.tile_pool(name="tps", bufs=TPP_BUFS, space="PSUM") as tpp,
          tc.tile_pool(name="ops", bufs=OPP_BUFS, space="PSUM") as opp,
      ):
        # ---- persistent tiles ----
        wq_t = pp.tile([128, 3, 2 * DH], BF16, tag="wq", name="wq")
        wkv_t = pp.tile([128, 6, 4 * DH], BF16, tag="wkv", name="wkv")
        wo_t = pp.tile([DH, 2, D_MODEL], BF16, tag="wo", name="wo")
        eye_t = pp.tile([128, 128], BF16, tag="eye", name="eye")
        xq_t = pp.tile([128, 3, S], BF16, tag="xq", name="xq")
        cond_t = pp.tile([128, 6, SK], BF16, tag="cond", name="cond")
        qT = [pp.tile([DH, S], BF16, tag=f"qT{h}", name=f"qT{h}") for h in range(2)]
        kT = [pp.tile([DH, SK], BF16, tag=f"kT{h}", name=f"kT{h}") for h in range(2)]
        # vaug[:, h, kc, 0:40] = V chunk, [.., 40] = 1.0 (denominator column)
        vaug = pp.tile([128, 2, NKC, DH + 1], BF16, tag="vaug", name="vaug")
        # for zero-initializing the AV psum bank (see attn_block)
        ones1 = pp.tile([1, 128], BF16, tag="ones1", name="ones1")
        zrow = pp.tile([1, NSC * (DH + 1)], BF16, tag="zrow", name="zrow")

        # ---- DMAs (sync engine queue; cond first, x after) ----
        nc.sync.dma_start(wkv_t[:], wkv_d[:])
        nc.sync.dma_start(wq_t[:], wq_d[:])
        nc.sync.dma_start(wo_t[:], wo_d[:])
        nc.sync.dma_start(eye_t[:], eye_d[:])
        for i in range(3):
            nc.sync.dma_start(cond_t[:, 2 * i:2 * i + 2, :],
                              cond_d[:, 2 * i:2 * i + 2, :])
        nc.sync.dma_start(xq_t[:], xq_d[:])
        with nc.allow_low_precision(reason="constant init"):
            nc.vector.memset(vaug[:, :, :, DH], 1.0)
            nc.vector.memset(ones1[:], 1.0)
            nc.vector.memset(zrow[:], 0.0)

        # ---- projection helpers (use the scores psum pool) ----
        def kq_proj_block(h, nb, w_tile, nch, src):
            """(q|k)T[h][:, nb*QB:(nb+1)*QB] = W_h^T @ src block."""
            ps = scp.tile([128, QB], F32, tag="sps", name="sps")
            for c in range(nch):
                n = XCH[c] if nch == 3 else 128
                nc.tensor.matmul(
                    ps[0:DH, :],
                    w_tile[0:n, c, h * DH:(h + 1) * DH],
                    src[0:n, c, nb * QB:(nb + 1) * QB],
                    start=(c == 0), stop=(c == nch - 1),
                )
            dst = (qT if nch == 3 else kT)[h][:, nb * QB:(nb + 1) * QB]
            with nc.allow_low_precision(reason="bf16 activations"):
                nc.scalar.copy(dst, ps[0:DH, :])

        def v_proj_block(skc):
            """vaug[:, :, skc, 0:40] = cond-chunk^T @ Wv (both heads)."""
            ps = scp.tile([128, QB], F32, tag="sps", name="sps")
            for c in range(6):
                nc.tensor.matmul(
                    ps[:, 0:2 * DH],
                    cond_t[:, c, skc * KC:(skc + 1) * KC],
                    wkv_t[:, c, 2 * DH:4 * DH],
                    start=(c == 0), stop=(c == 5),
                )
            with nc.allow_low_precision(reason="bf16 activations"):
                nc.vector.tensor_copy(vaug[:, :, skc, 0:DH], ps[:, 0:2 * DH])

        # ---- attention block ----
        W = SCP_BUFS  # scores psum banks in flight

        def attn_block(h, qb, outT_tile, extra_cb=None):
            q_sl = qT[h][:, qb * QB:(qb + 1) * QB]
            p_t = ptp.tile([128, NKC, QB], BF16, tag="pt", name="pt")
            # full-bank tile: matmul start=True zeroes a whole 2KB region, so
            # the 4 interleaved per-sub-chunk accumulation groups must own the
            # bank exclusively and are seeded by ONE zeroing matmul instead.
            av = avp.tile([128, NSC, 128], F32, tag="av", name="av")
            nc.tensor.matmul(av[:, :, 0:DH + 1], ones1[:], zrow[:],
                             start=True, stop=False, skip_group_check=True)
            sc_tiles = {}

            def scores(kc):
                sp = scp.tile([128, QB], F32, tag="sps", name="sps")
                sc_tiles[kc] = sp
                nc.tensor.matmul(sp[:], kT[h][:, kc * KC:(kc + 1) * KC], q_sl,
                                 start=True, stop=True)

            for kc in range(W):
                scores(kc)
            for kc in range(NKC):
                if kc + W < NKC:
                    scores(kc + W)
                # drip-fed extra PE work must be emitted BEFORE the AV matmul
                # that may depend on it (V projection writes vaug)
                if extra_cb is not None:
                    extra_cb(kc)
                sp = sc_tiles.pop(kc)
                dst = p_t[:, kc, :]
                e = ENG[kc]
                if e == "A":
                    nc.scalar.activation(dst, sp[:], EXP, scale=float(SCALE))
                else:
                    eng = nc.vector if e == "D" else nc.gpsimd
                    eng.tensor_scalar(dst.bitcast(I16), sp[:],
                                      float(SCH_A), float(SCH_B),
                                      AluOpType.mult, AluOpType.add)
                for sc in range(NSC):
                    nc.tensor.matmul(
                        av[:, sc, 0:DH + 1],
                        p_t[:, kc, sc * 128:(sc + 1) * 128],
                        vaug[:, h, kc, :],
                        start=False, stop=(kc == NKC - 1),
                        skip_group_check=True,
                    )

            # normalize (per-query 1/denominator) + transpose to [40, 512]
            recip = rcp.tile([128, NSC], F32, tag="rcp", name="rcp")
            nc.vector.reciprocal(recip[:], av[:, :, DH])
            o_nat = onp.tile([128, NSC, DH], BF16, tag="onat", name="onat")
            t_ps = tpp.tile([DH, QB], BF16, tag="tps", name="tps")
            for sc in range(NSC):
                # ACT Copy-with-scale: out = in * recip[partition]
                with nc.allow_low_precision(reason="normalized attn out"):
                    nc.scalar.activation(o_nat[:, sc, :], av[:, sc, 0:DH],
                                         COPYF, scale=recip[:, sc:sc + 1])
                nc.tensor.transpose(t_ps[:, sc * 128:(sc + 1) * 128],
                                    o_nat[:, sc, :], eye_t[:])
            with nc.allow_low_precision(reason="bf16 activations"):
                nc.vector.tensor_copy(outT_tile[:, h, :], t_ps[:])

        def oproj(qb, outT_tile):
            for st in range(NSC):
                o_ps = opp.tile([128, D_MODEL], F32, tag="ops", name="ops")
                for h in range(2):
                    nc.tensor.matmul(
                        o_ps[:], outT_tile[:, h, st * 128:(st + 1) * 128],
                        wo_t[:, h, :], start=(h == 0), stop=(h == 1),
                    )
                o_sb = obp.tile([128, D_MODEL], BF16, tag="osb", name="osb")
                with nc.allow_low_precision(reason="bf16 partial output"):
                    nc.scalar.copy(o_sb[:], o_ps[:])
                row = qb * QB + st * 128
                nc.sync.dma_start(out_d[row:row + 128, :], o_sb[:])

        # ---- emission schedule ----
        # prologue: K and Q of head 0 fully, V chunks 0..VA-1.
        VA = 4  # V-projection lookahead inside block (0, 0)
        for nb in range(NQB):
            kq_proj_block(0, nb, wkv_t, 6, cond_t)
        for nb in range(NQB):
            kq_proj_block(0, nb, wq_t, 3, xq_t)
        for skc in range(VA):
            v_proj_block(skc)

        # head-1 K/Q drip-fed into the head-0 attention blocks:
        pending = []
        for nb in range(NQB):          # K head1
            pending.append(lambda nb=nb: kq_proj_block(1, nb, wkv_t, 6, cond_t))
        for nb in range(NQB):          # Q head1
            pending.append(lambda nb=nb: kq_proj_block(1, nb, wq_t, 3, xq_t))

        def v_drip(kc):
            # keep V emission VA chunks ahead of the consuming AV matmul
            if kc + VA < NKC:
                v_proj_block(kc + VA)

        def gen_drip(kc):
            if pending and kc % 5 == 0:
                pending.pop(0)()

        # interleaved (h, qb) order: h1 starts once its K/Q have drained.
        order = [(0, 0), (0, 1), (0, 2), (0, 3), (0, 4),
                 (1, 0), (0, 5), (1, 1), (0, 6), (1, 2), (0, 7),
                 (1, 3), (1, 4), (1, 5), (1, 6), (1, 7)]
        outT_tiles = {}
        done_h = {}
        for i, (h, qb) in enumerate(order):
            if qb not in outT_tiles:
                outT_tiles[qb] = otp.tile([DH, 2, QB], BF16, tag="outT",
                                          name="outT")
            cb = v_drip if i == 0 else (gen_drip if pending else None)
            attn_block(h, qb, outT_tiles[qb], cb)
            done_h[qb] = done_h.get(qb, 0) + 1
            if done_h[qb] == 2:
                oproj(qb, outT_tiles.pop(qb))

    nc.compile()
    return nc


def _get_nc():
    if "nc" not in _CACHE:
        _CACHE["nc"] = _build_nc()
    return _CACHE["nc"]


def kernel(x, cond, Wq, Wk, Wv, Wo, bo, _collect_results=None):
    import ml_dtypes

    BF = ml_dtypes.bfloat16
    x = np.asarray(x, dtype=np.float32)
    cond = np.asarray(cond, dtype=np.float32)
    Wq = np.asarray(Wq, dtype=np.float32)
    Wk = np.asarray(Wk, dtype=np.float32)
    Wv = np.asarray(Wv, dtype=np.float32)
    Wo = np.asarray(Wo, dtype=np.float32)
    bo = np.asarray(bo, dtype=np.float32)

    from concourse.bass_utils import run_bass_kernel_spmd

    nc = _get_nc()

    eye = np.eye(128, dtype=np.float32).astype(BF)

    def pack_feat(a, nch):
        # [F, N] -> [128, nch, N], zero-padded partitions
        f, n = a.shape
        out = np.zeros((128, nch, n), dtype=BF)
        for c in range(nch):
            r = min(128, f - c * 128)
            out[:r, c, :] = a[c * 128:c * 128 + r, :].astype(BF)
        return out

    in_maps = []
    for c in range(NCORES):
        b, h0 = c // 4, 2 * (c % 4)
        wq2 = Wq[:, h0 * DH:(h0 + 2) * DH]                      # [320, 80]
        wk2 = Wk[:, h0 * DH:(h0 + 2) * DH]                      # [768, 80]
        wv2 = Wv[:, h0 * DH:(h0 + 2) * DH]                      # [768, 80]
        wkv = np.concatenate([wk2, wv2], axis=1)                # [768, 160]
        wo2 = Wo[h0 * DH:(h0 + 2) * DH, :]                      # [80, 320]
        in_maps.append({
            "xq": pack_feat(x[b].T, 3),
            "cond6": pack_feat(cond[b].T, 6),
            "wq": pack_feat(wq2, 3),
            "wkv": pack_feat(wkv, 6),
            "wo": np.ascontiguousarray(
                wo2.reshape(2, DH, D_MODEL).transpose(1, 0, 2)).astype(BF),
            "eyed": eye,
        })

    kw = _CACHE.pop("run_kwargs", {})
    res = run_bass_kernel_spmd(nc, in_maps, core_ids=list(range(NCORES)), **kw)
    if _collect_results is not None:
        _collect_results.append(res)
    outs = [np.asarray(r["out"], dtype=np.float32) for r in res.results]
    full = np.stack([
        outs[0] + outs[1] + outs[2] + outs[3],
        outs[4] + outs[5] + outs[6] + outs[7],
    ]).astype(np.float32)
    return full + bo[None, None, :]
